# revision 1
# baseline (speedup 1.0000x reference)
import sys

sys.path.insert(0, "/opt/trn_rl_repo")

import numpy as np
import ml_dtypes
from contextlib import ExitStack

import concourse.bass as bass
import concourse.tile as tile
from concourse import bacc
from concourse import mybir
from concourse.bass_utils import run_bass_kernel_spmd

B, S, HID = 2, 2048, 1024
NH, NKV, HD = 16, 4, 64
P = 128
NK = HID // P
NQC = S // 512
NST = S // P
QH = NH // NKV
FEAT = QH * HD
MASK_NEG = -1e9

F32 = mybir.dt.float32
F32R = mybir.dt.float32r
BF16 = mybir.dt.bfloat16


def _r(ap):
    return ap.bitcast(F32R)


def _pin_act_tables():
    import concourse.hw_specs as hw_specs
    import concourse.bacc as bacc_mod
    real = hw_specs.get_activation_tables

    def pinned(arch):
        tabs = dict(real(arch))
        return {name: (funcs if name == "natural_log_exp_and_others" else set())
                for name, funcs in tabs.items()}

    bacc_mod.get_activation_tables = pinned


def build_program():
    _pin_act_tables()
    nc = bacc.Bacc("TRN2", target_bir_lowering=False, debug=False)

    d_xT = nc.dram_tensor("xT", [HID, S], F32R, kind="ExternalInput").ap()
    d_wqT = nc.dram_tensor("wqT", [HID, FEAT], F32R, kind="ExternalInput").ap()
    d_wkvT = nc.dram_tensor("wkvT", [HID, 2 * HD], F32R, kind="ExternalInput").ap()
    d_woT = nc.dram_tensor("woT", [FEAT, HID], F32R, kind="ExternalInput").ap()
    d_cosT = nc.dram_tensor("cosT", [P, S], F32, kind="ExternalInput").ap()
    d_sinT = nc.dram_tensor("sinT", [P, S], F32, kind="ExternalInput").ap()
    d_r128 = nc.dram_tensor("r128", [P, P], F32R, kind="ExternalInput").ap()
    d_rdup = nc.dram_tensor("rdup", [HD, P], F32R, kind="ExternalInput").ap()
    d_idup = nc.dram_tensor("idup", [HD, P], F32R, kind="ExternalInput").ap()
    d_ident = nc.dram_tensor("ident", [P, HD], F32R, kind="ExternalInput").ap()
    d_ishift = nc.dram_tensor("ishift", [HD, P], F32R, kind="ExternalInput").ap()
    d_tri = nc.dram_tensor("tri", [P, P], BF16, kind="ExternalInput").ap()
    d_ones1 = nc.dram_tensor("ones1c", [1, HD], F32R, kind="ExternalInput").ap()
    d_onesv = nc.dram_tensor("onesv", [P, NST], F32R, kind="ExternalInput").ap()
    d_i128b = nc.dram_tensor("i128b", [P, P], BF16, kind="ExternalInput").ap()
    d_out = nc.dram_tensor("outp", [S, HID], F32, kind="ExternalOutput").ap()

    with tile.TileContext(nc) as tc, ExitStack() as ctx, \
            nc.allow_low_precision(reason="float32r is bit-identical to fp32"):
        consts = ctx.enter_context(tc.tile_pool(name="consts", bufs=1))
        main = ctx.enter_context(tc.tile_pool(name="main", bufs=1))

        wq_sb = consts.tile([P, NK, FEAT], F32R)
        nc.scalar.dma_start(wq_sb[:], d_wqT.rearrange("(ko p) m -> p ko m", p=P))
        wkv_sb = consts.tile([P, NK, 2 * HD], F32R)
        nc.scalar.dma_start(wkv_sb[:], d_wkvT.rearrange("(ko p) m -> p ko m", p=P))
        wo_sb = consts.tile([P, 2, HID], F32R)
        nc.gpsimd.dma_start(wo_sb[:], d_woT.rearrange("(ko p) m -> p ko m", p=P))
        cos_sb = consts.tile([P, S], F32)
        nc.scalar.dma_start(cos_sb[:], d_cosT)
        sin_sb = consts.tile([P, S], F32)
        nc.gpsimd.dma_start(sin_sb[:], d_sinT)
        r128_sb = consts.tile([P, P], F32R)
        nc.sync.dma_start(r128_sb[:], d_r128)
        rdup_sb = consts.tile([HD, P], F32R)
        nc.sync.dma_start(rdup_sb[:], d_rdup)
        idup_sb = consts.tile([HD, P], F32R)
        nc.sync.dma_start(idup_sb[:], d_idup)
        ident_sb = consts.tile([P, HD], F32R)
        nc.sync.dma_start(ident_sb[:], d_ident)
        ishift_sb = consts.tile([HD, P], F32R)
        nc.sync.dma_start(ishift_sb[:], d_ishift)
        tri_sb = consts.tile([P, P], BF16)
        nc.sync.dma_start(tri_sb[:], d_tri)
        i128b_sb = consts.tile([P, P], BF16)
        nc.sync.dma_start(i128b_sb[:], d_i128b)

        qpt = main.tile([P, 2, S], F32R)
        kpt = main.tile([P, S], F32R)
        vaug = main.tile([P, NST, HD + 1], F32R)
        attnT = main.tile([P, 2, S], F32R)

        nc.sync.dma_start(vaug[:, :, HD:HD + 1], d_onesv)
        ones1 = consts.tile([1, HD], F32R)
        nc.sync.dma_start(ones1[:], d_ones1)

        with ExitStack() as pw:
            wps = pw.enter_context(tc.tile_pool(name="wps", bufs=1, space="PSUM"))
            wp = wps.tile([P, P], F32, tag="warm")
            for _ in range(48):
                nc.tensor.matmul(wp[:], i128b_sb[:], i128b_sb[:],
                                 start=True, stop=True)

        with ExitStack() as p1:
            xpool = p1.enter_context(tc.tile_pool(name="xt", bufs=8))
            rawp = p1.enter_context(tc.tile_pool(name="raw", bufs=3))
            tmpp = p1.enter_context(tc.tile_pool(name="ropetmp", bufs=4))
            pj = p1.enter_context(tc.tile_pool(name="pjps", bufs=3, space="PSUM"))
            rps = p1.enter_context(tc.tile_pool(name="rops", bufs=2, space="PSUM"))
            vtp = p1.enter_context(tc.tile_pool(name="vtps", bufs=2, space="PSUM"))

            for n in range(NQC):
                c0 = n * 512
                ps_q0 = pj.tile([P, 512], F32, tag="pj")
                ps_q1 = pj.tile([P, 512], F32, tag="pj")
                ps_kv = pj.tile([P, 512], F32, tag="pj")
                for k in range(NK):
                    xt = xpool.tile([P, 512], F32R)
                    eng = nc.sync if k % 2 == 0 else nc.gpsimd
                    eng.dma_start(xt[:], d_xT[k * P:(k + 1) * P, c0:c0 + 512])
                    nc.tensor.matmul(ps_q0[:], _r(wq_sb[:, k, 0:P]), _r(xt[:]),
                                     start=(k == 0), stop=(k == NK - 1))
                    nc.tensor.matmul(ps_q1[:], _r(wq_sb[:, k, P:FEAT]), _r(xt[:]),
                                     start=(k == 0), stop=(k == NK - 1))
                    nc.tensor.matmul(ps_kv[:], _r(wkv_sb[:, k, :]), _r(xt[:]),
                                     start=(k == 0), stop=(k == NK - 1))
                cs = cos_sb[:, c0:c0 + 512]
                sn = sin_sb[:, c0:c0 + 512]
                for m, ps_q in ((0, ps_q0), (1, ps_q1)):
                    qraw = rawp.tile([P, 512], F32R, tag="qraw")
                    nc.vector.tensor_copy(qraw[:], ps_q[:])
                    ps_qr = rps.tile([P, 512], F32, tag="rot")
                    nc.tensor.matmul(ps_qr[:], _r(r128_sb[:]), _r(qraw[:]),
                                     start=True, stop=True)
                    t1 = tmpp.tile([P, 512], F32, tag="t1")
                    nc.vector.tensor_mul(t1[:], qraw[:], cs)
                    t2 = tmpp.tile([P, 512], F32, tag="t2")
                    nc.vector.tensor_mul(t2[:], ps_qr[:], sn)
                    nc.gpsimd.tensor_add(qpt[:, m, c0:c0 + 512], t1[:], t2[:])
                kvraw = rawp.tile([P, 512], F32R, tag="kvraw")
                nc.vector.tensor_copy(kvraw[:], ps_kv[:])
                ps_k2 = rps.tile([P, 512], F32, tag="rot")
                nc.tensor.matmul(ps_k2[:], _r(idup_sb[:]), _r(kvraw[0:HD, :]),
                                 start=True, stop=True)
                ps_kr = rps.tile([P, 512], F32, tag="rot")
                nc.tensor.matmul(ps_kr[:], _r(rdup_sb[:]), _r(kvraw[0:HD, :]),
                                 start=True, stop=True)
                t1 = tmpp.tile([P, 512], F32, tag="t1")
                nc.vector.tensor_mul(t1[:], ps_k2[:], cs)
                t2 = tmpp.tile([P, 512], F32, tag="t2")
                nc.vector.tensor_mul(t2[:], ps_kr[:], sn)
                nc.gpsimd.tensor_add(kpt[:, c0:c0 + 512], t1[:], t2[:])
                for tt in range(4):
                    st = 4 * n + tt
                    ps_v = vtp.tile([P, HD], F32R, tag="vt")
                    nc.tensor.transpose(ps_v[:], kvraw[HD:P, tt * P:(tt + 1) * P],
                                        ident_sb[HD:P, :])
                    nc.vector.tensor_copy(vaug[:, st, 0:HD], ps_v[:])

        with ExitStack() as p2:
            ptp = p2.enter_context(tc.tile_pool(name="pt", bufs=8))
            recp = p2.enter_context(tc.tile_pool(name="rec", bufs=6))
            oddp = p2.enter_context(tc.tile_pool(name="odd", bufs=2))
            scps = p2.enter_context(tc.tile_pool(name="scps", bufs=2, space="PSUM"))
            bcps = p2.enter_context(tc.tile_pool(name="bcps", bufs=1, space="PSUM"))
            pvps = p2.enter_context(tc.tile_pool(name="pvps", bufs=2, space="PSUM"))
            rpps = p2.enter_context(tc.tile_pool(name="rpps", bufs=1, space="PSUM"))

            for m in range(2):
                for j in range(NQC):
                    c0 = j * 512
                    T = 4 * j + 4
                    pvh = [pvps.tile([P, 512], F32, tag="pv", name=f"pv{m}{j}{h2}")
                           for h2 in (0, 1)]
                    for t in range(T):
                        r = t - 4 * j
                        lo = P * r if r >= 0 else 0
                        sc = scps.tile([P, 1024], F32, tag="sc")
                        pt = ptp.tile([P, 1024], F32R, tag="ptt", name=f"pt{m}{j}{t}")
                        for h2 in (0, 1):
                            half = h2 * HD
                            kl = kpt[half:half + HD, t * P:(t + 1) * P]
                            ql = qpt[half:half + HD, m, c0 + lo:c0 + 512]
                            scv = sc[:, h2 * 512:h2 * 512 + 512]
                            if r >= 0:
                                nc.tensor.matmul(
                                    scv[:, lo:512], _r(kl), _r(ql),
                                    start=True, stop=False,
                                    skip_group_check=True)
                                nc.tensor.matmul(
                                    scv[:, lo:lo + P], i128b_sb[:], tri_sb[:],
                                    start=False, stop=True,
                                    skip_group_check=True)
                            else:
                                nc.tensor.matmul(scv[:], _r(kl), _r(ql),
                                                 start=True, stop=True,
                                                 skip_group_check=True)
                        if lo == 0:
                            nc.scalar.activation(
                                pt[:], sc[:],
                                mybir.ActivationFunctionType.Exp, scale=0.125)
                        else:
                            for h2 in (0, 1):
                                o2 = h2 * 512
                                nc.scalar.activation(
                                    pt[:, o2 + lo:o2 + 512],
                                    sc[:, o2 + lo:o2 + 512],
                                    mybir.ActivationFunctionType.Exp,
                                    scale=0.125)
                        for h2 in (0, 1):
                            nc.tensor.matmul(
                                pvh[h2][0:HD + 1, lo:512],
                                _r(vaug[:, t, :]),
                                _r(pt[:, h2 * 512 + lo:h2 * 512 + 512]),
                                start=(t == 0), stop=(t == T - 1),
                                skip_group_check=True)
                    for h2 in (0, 1):
                        pv = pvh[h2]
                        lnt = recp.tile([1, 512], F32, tag="lnt")
                        nc.scalar.activation(lnt[:], pv[HD:HD + 1, :],
                                             mybir.ActivationFunctionType.Ln)
                        recr = recp.tile([1, 512], F32R, tag="recr")
                        nc.scalar.activation(recr[:], lnt[:],
                                             mybir.ActivationFunctionType.Exp,
                                             scale=-1.0)
                        rec_b = bcps.tile([HD, 512], F32, tag="recb")
                        nc.tensor.matmul(rec_b[:], ones1[:], recr[:],
                                         start=True, stop=True)
                        rec_s = recp.tile([HD, 512], F32, tag="recs")
                        nc.vector.tensor_copy(rec_s[:], rec_b[:])
                        if h2 == 0:
                            nc.vector.tensor_mul(attnT[0:HD, m, c0:c0 + 512],
                                                 pv[0:HD, :], rec_s[:])
                        else:
                            tmp = oddp.tile([HD, 512], F32R, tag="oddt")
                            nc.vector.tensor_mul(tmp[:], pv[0:HD, :], rec_s[:])
                            rp = rpps.tile([P, 512], F32, tag="rp")
                            nc.tensor.matmul(rp[:], ishift_sb[:], tmp[:],
                                             start=True, stop=True)
                            nc.vector.tensor_copy(attnT[HD:P, m, c0:c0 + 512],
                                                  rp[HD:P, :])

        with ExitStack() as p3:
            osb = p3.enter_context(tc.tile_pool(name="osb", bufs=4))
            ops = p3.enter_context(tc.tile_pool(name="ops", bufs=3, space="PSUM"))
            for st in range(NST):
                for nn in range(2):
                    po = ops.tile([P, 512], F32, tag="po")
                    for m in range(2):
                        nc.tensor.matmul(po[:],
                                         _r(attnT[:, m, st * P:(st + 1) * P]),
                                         _r(wo_sb[:, m, nn * 512:(nn + 1) * 512]),
                                         start=(m == 0), stop=(m == 1))
                    ot = osb.tile([P, 512], F32, tag="ot")
                    nc.vector.tensor_copy(ot[:], po[:])
                    nc.sync.dma_start(
                        d_out[st * P:(st + 1) * P, nn * 512:(nn + 1) * 512],
                        ot[:])

    nc.compile()
    return nc


def make_consts():
    r128 = np.zeros((P, P), np.float32)
    for mm in range(P):
        hh, dd = mm // HD, mm % HD
        if dd < HD // 2:
            r128[hh * HD + dd + HD // 2, mm] = -1.0
        else:
            r128[hh * HD + dd - HD // 2, mm] = 1.0
    rdup = np.zeros((HD, P), np.float32)
    idup = np.zeros((HD, P), np.float32)
    for mm in range(P):
        dd = mm % HD
        idup[dd, mm] = 1.0
        if dd < HD // 2:
            rdup[dd + HD // 2, mm] = -1.0
        else:
            rdup[dd - HD // 2, mm] = 1.0
    ident = np.zeros((P, HD), np.float32)
    ident[HD:P, :] = np.eye(HD)
    ishift = np.zeros((HD, P), np.float32)
    for kk in range(HD):
        ishift[kk, kk + HD] = 1.0
    tri = np.where(np.arange(P)[:, None] <= np.arange(P)[None, :], 0.0,
                   MASK_NEG).astype(ml_dtypes.bfloat16)
    i128b = np.eye(P).astype(ml_dtypes.bfloat16)
    return dict(r128=r128, rdup=rdup, idup=idup, ident=ident, ishift=ishift,
                tri=tri, i128b=i128b, ones1c=np.ones((1, HD), np.float32),
                onesv=np.ones((P, NST), np.float32))


_PROG = None


def kernel(x, cos, sin, wq, wk, wv, wo):
    global _PROG
    x = np.asarray(x, np.float32)
    cos = np.asarray(cos, np.float32)
    sin = np.asarray(sin, np.float32)
    wq = np.asarray(wq, np.float32)
    wk = np.asarray(wk, np.float32)
    wv = np.asarray(wv, np.float32)
    wo = np.asarray(wo, np.float32)

    consts = make_consts()
    cosT = np.ascontiguousarray(np.vstack([cos.T, cos.T]))
    sinT = np.ascontiguousarray(np.vstack([sin.T, sin.T]))

    in_maps = []
    for core in range(8):
        b, g = core // NKV, core % NKV
        xT = np.ascontiguousarray(x[b].T)
        wqT = np.ascontiguousarray(wq[g * FEAT:(g + 1) * FEAT, :].T)
        wkvT = np.ascontiguousarray(
            np.concatenate([wk[g * HD:(g + 1) * HD, :],
                            wv[g * HD:(g + 1) * HD, :]], axis=0).T)
        woT = np.ascontiguousarray(wo[:, g * FEAT:(g + 1) * FEAT].T)
        in_maps.append(dict(xT=xT, wqT=wqT, wkvT=wkvT, woT=woT,
                            cosT=cosT, sinT=sinT, **consts))

    if _PROG is None:
        _PROG = build_program()
    res = run_bass_kernel_spmd(_PROG, in_maps, core_ids=list(range(8)))

    out = np.zeros((B, S, HID), np.float32)
    for core in range(8):
        out[core // NKV] += res.results[core]["outp"]
    return out


if __name__ == "__main__":
    rng = np.random.default_rng(0)
    ins = dict(
        x=rng.standard_normal((B, S, HID), np.float32),
        cos=rng.random((S, HD), np.float32),
        sin=rng.random((S, HD), np.float32),
        wq=rng.standard_normal((HID, HID), np.float32) * HID ** -0.5,
        wk=rng.standard_normal((NKV * HD, HID), np.float32) * HID ** -0.5,
        wv=rng.standard_normal((NKV * HD, HID), np.float32) * HID ** -0.5,
        wo=rng.standard_normal((HID, HID), np.float32) * HID ** -0.5,
    )
    out = kernel(**ins)
    print("kernel ran, out shape", out.shape, "mean", float(np.abs(out).mean()))



# revision 5
# speedup vs baseline: 1.1544x; 1.1544x over previous
import sys

sys.path.insert(0, "/opt/trn_rl_repo")

import numpy as np
import ml_dtypes
from contextlib import ExitStack

import concourse.bass as bass
import concourse.tile as tile
from concourse import bacc
from concourse import mybir
from concourse.bass_utils import run_bass_kernel_spmd

B, S, HID = 2, 2048, 1024
NH, NKV, HD = 16, 4, 64
P = 128
NK = HID // P
NQC = S // 512
NST = S // P
QH = NH // NKV
FEAT = QH * HD
MASK_NEG = -1e9

F32 = mybir.dt.float32
F32R = mybir.dt.float32r
BF16 = mybir.dt.bfloat16


def _r(ap):
    return ap.bitcast(F32R)


def _pin_act_tables():
    import concourse.hw_specs as hw_specs
    import concourse.bacc as bacc_mod
    real = hw_specs.get_activation_tables

    def pinned(arch):
        tabs = dict(real(arch))
        return {name: (funcs if name == "natural_log_exp_and_others" else set())
                for name, funcs in tabs.items()}

    bacc_mod.get_activation_tables = pinned


def build_program():
    _pin_act_tables()
    nc = bacc.Bacc("TRN2", target_bir_lowering=False, debug=False)

    d_xT = nc.dram_tensor("xT", [HID, S], BF16, kind="ExternalInput").ap()
    d_wqT = nc.dram_tensor("wqT", [HID, FEAT], BF16, kind="ExternalInput").ap()
    d_wkvT = nc.dram_tensor("wkvT", [HID, 2 * HD], BF16,
                            kind="ExternalInput").ap()
    d_woT = nc.dram_tensor("woT", [FEAT, HID], BF16, kind="ExternalInput").ap()
    d_cosT = nc.dram_tensor("cosT", [P, S], BF16, kind="ExternalInput").ap()
    d_sinT = nc.dram_tensor("sinT", [P, S], BF16, kind="ExternalInput").ap()
    d_r128 = nc.dram_tensor("r128", [P, P], BF16, kind="ExternalInput").ap()
    d_rdup = nc.dram_tensor("rdup", [HD, P], BF16, kind="ExternalInput").ap()
    d_idup = nc.dram_tensor("idup", [HD, P], BF16, kind="ExternalInput").ap()
    d_ident = nc.dram_tensor("ident", [P, HD], BF16, kind="ExternalInput").ap()
    d_ishift = nc.dram_tensor("ishift", [HD, P], BF16,
                              kind="ExternalInput").ap()
    d_tri = nc.dram_tensor("tri", [P, P], BF16, kind="ExternalInput").ap()
    d_ones1 = nc.dram_tensor("ones1c", [1, HD], F32R, kind="ExternalInput").ap()
    d_onesv = nc.dram_tensor("onesv", [P, NST], BF16, kind="ExternalInput").ap()
    d_i128b = nc.dram_tensor("i128b", [P, P], BF16, kind="ExternalInput").ap()
    d_out = nc.dram_tensor("outp", [S, HID], BF16, kind="ExternalOutput").ap()

    with tile.TileContext(nc) as tc, ExitStack() as ctx, \
            nc.allow_low_precision(reason="bf16 compute fits 2e-2 tolerance"):
        consts = ctx.enter_context(tc.tile_pool(name="consts", bufs=1))
        main = ctx.enter_context(tc.tile_pool(name="main", bufs=1))

        wq_sb = consts.tile([P, NK, FEAT], BF16)
        nc.scalar.dma_start(wq_sb[:], d_wqT.rearrange("(ko p) m -> p ko m", p=P))
        wkv_sb = consts.tile([P, NK, 2 * HD], BF16)
        nc.scalar.dma_start(wkv_sb[:],
                            d_wkvT.rearrange("(ko p) m -> p ko m", p=P))
        wo_sb = consts.tile([P, 2, HID], BF16)
        nc.gpsimd.dma_start(wo_sb[:], d_woT.rearrange("(ko p) m -> p ko m", p=P))
        cos_sb = consts.tile([P, S], BF16)
        nc.scalar.dma_start(cos_sb[:], d_cosT)
        sin_sb = consts.tile([P, S], BF16)
        nc.gpsimd.dma_start(sin_sb[:], d_sinT)
        r128_sb = consts.tile([P, P], BF16)
        nc.sync.dma_start(r128_sb[:], d_r128)
        rdup_sb = consts.tile([HD, P], BF16)
        nc.sync.dma_start(rdup_sb[:], d_rdup)
        idup_sb = consts.tile([HD, P], BF16)
        nc.sync.dma_start(idup_sb[:], d_idup)
        ident_sb = consts.tile([P, HD], BF16)
        nc.sync.dma_start(ident_sb[:], d_ident)
        ishift_sb = consts.tile([HD, P], BF16)
        nc.sync.dma_start(ishift_sb[:], d_ishift)
        tri_sb = consts.tile([P, P], BF16)
        nc.sync.dma_start(tri_sb[:], d_tri)
        i128b_sb = consts.tile([P, P], BF16)
        nc.sync.dma_start(i128b_sb[:], d_i128b)
        ones1 = consts.tile([1, HD], F32R)
        nc.sync.dma_start(ones1[:], d_ones1)

        xsb = main.tile([P, NK, S], BF16)
        dma_engs = [nc.sync, nc.gpsimd, nc.scalar]
        for n in range(NQC):
            for k in range(NK):
                eng = dma_engs[(n * NK + k) % 3]
                eng.dma_start(xsb[:, k, n * 512:(n + 1) * 512],
                              d_xT[k * P:(k + 1) * P, n * 512:(n + 1) * 512])

        qpt = main.tile([P, 2, S], BF16)
        kpt = main.tile([P, S], BF16)
        vaug = main.tile([P, NST, HD + 1], BF16)
        attnT = main.tile([P, 2, S], BF16)

        nc.sync.dma_start(vaug[:, :, HD:HD + 1], d_onesv)

        scps = ctx.enter_context(tc.tile_pool(name="scps", bufs=2, space="PSUM"))
        pvps = ctx.enter_context(tc.tile_pool(name="pvps", bufs=1, space="PSUM"))
        util = ctx.enter_context(tc.tile_pool(name="util", bufs=2, space="PSUM"))

        rawp = ctx.enter_context(tc.tile_pool(name="rawp", bufs=3))
        tmpp = ctx.enter_context(tc.tile_pool(name="tmpp", bufs=2))
        ptp = ctx.enter_context(tc.tile_pool(name="ptp", bufs=4))
        recp = ctx.enter_context(tc.tile_pool(name="recp", bufs=2))
        outp = ctx.enter_context(tc.tile_pool(name="outp", bufs=3))

        for i in range(16):
            wp = util.tile([P, P], F32, tag="ut", name=f"warm{i}")
            nc.tensor.matmul(wp[:], i128b_sb[:], i128b_sb[:],
                             start=True, stop=True)


        def emit_proj_q(n, m):
            c0 = n * 512
            ps = util.tile([P, 512], F32, tag="ut", name=f"pjq{n}{m}")
            for k in range(NK):
                nc.tensor.matmul(ps[:], wq_sb[:, k, m * P:(m + 1) * P],
                                 xsb[:, k, c0:c0 + 512],
                                 start=(k == 0), stop=(k == NK - 1))
            raw = rawp.tile([P, 512], BF16, tag="raw", name=f"qraw{n}{m}")
            nc.vector.tensor_copy(raw[:], ps[:])
            psr = util.tile([P, 512], F32, tag="ut", name=f"pjqr{n}{m}")
            nc.tensor.matmul(psr[:], r128_sb[:], raw[:], start=True, stop=True)
            cs = cos_sb[:, c0:c0 + 512]
            sn = sin_sb[:, c0:c0 + 512]
            t1 = tmpp.tile([P, 512], BF16, tag="t1", name=f"t1q{n}{m}")
            nc.vector.tensor_mul(t1[:], raw[:], cs)
            t2 = tmpp.tile([P, 512], BF16, tag="t2", name=f"t2q{n}{m}")
            nc.vector.tensor_mul(t2[:], psr[:], sn)
            nc.gpsimd.tensor_add(qpt[:, m, c0:c0 + 512], t1[:], t2[:])

        def emit_proj_kv(n):
            c0 = n * 512
            ps = util.tile([P, 512], F32, tag="ut", name=f"pjkv{n}")
            for k in range(NK):
                nc.tensor.matmul(ps[:], wkv_sb[:, k, :],
                                 xsb[:, k, c0:c0 + 512],
                                 start=(k == 0), stop=(k == NK - 1))
            raw = rawp.tile([P, 512], BF16, tag="raw", name=f"kvraw{n}")
            nc.vector.tensor_copy(raw[:], ps[:])
            cs = cos_sb[:, c0:c0 + 512]
            sn = sin_sb[:, c0:c0 + 512]
            psk2 = util.tile([P, 512], F32, tag="ut", name=f"pjk2{n}")
            nc.tensor.matmul(psk2[:], idup_sb[:], raw[0:HD, :],
                             start=True, stop=True)
            t1 = tmpp.tile([P, 512], BF16, tag="t1", name=f"t1k{n}")
            nc.vector.tensor_mul(t1[:], psk2[:], cs)
            pskr = util.tile([P, 512], F32, tag="ut", name=f"pjkr{n}")
            nc.tensor.matmul(pskr[:], rdup_sb[:], raw[0:HD, :],
                             start=True, stop=True)
            t2 = tmpp.tile([P, 512], BF16, tag="t2", name=f"t2k{n}")
            nc.vector.tensor_mul(t2[:], pskr[:], sn)
            nc.gpsimd.tensor_add(kpt[:, c0:c0 + 512], t1[:], t2[:])
            for tt in range(4):
                st = 4 * n + tt
                psv = util.tile([P, HD], BF16, tag="ut", name=f"vt{n}{tt}")
                nc.tensor.transpose(psv[:], raw[HD:P, tt * P:(tt + 1) * P],
                                    ident_sb[HD:P, :])
                nc.vector.tensor_copy(vaug[:, st, 0:HD], psv[:])

        def emit_outproj(st, nn):
            po = util.tile([P, 512], F32, tag="ut", name=f"po{st}{nn}")
            for m in range(2):
                nc.tensor.matmul(po[:], attnT[:, m, st * P:(st + 1) * P],
                                 wo_sb[:, m, nn * 512:(nn + 1) * 512],
                                 start=(m == 0), stop=(m == 1))
            ot = outp.tile([P, 512], BF16, tag="ot", name=f"ot{st}{nn}")
            nc.vector.tensor_copy(ot[:], po[:])
            nc.sync.dma_start(
                d_out[st * P:(st + 1) * P, nn * 512:(nn + 1) * 512], ot[:])

        fillers = []

        def pop_filler(k=1):
            for _ in range(k):
                if fillers:
                    fillers.pop(0)()

        def emit_attn(m, j):
            c0 = j * 512
            T = 4 * j + 4
            pv = pvps.tile([HD + 1, 2, 512], F32, tag="pv", name=f"pv{m}{j}")
            for t in range(T):
                r = t - 4 * j
                lo = P * r if r >= 0 else 0
                sc = scps.tile([P, 2, 512], F32, tag="sc", name=f"sc{m}{j}{t}")
                pt = ptp.tile([P, 2, 512], BF16, tag="pt", name=f"pt{m}{j}{t}")
                for h2 in (0, 1):
                    half = h2 * HD
                    kl = kpt[half:half + HD, t * P:(t + 1) * P]
                    ql = qpt[half:half + HD, m, c0 + lo:c0 + 512]
                    if r >= 0:
                        nc.tensor.matmul(sc[:, h2, lo:512], kl, ql,
                                         start=True, stop=False,
                                         skip_group_check=True)
                        nc.tensor.matmul(sc[:, h2, lo:lo + P], i128b_sb[:],
                                         tri_sb[:], start=False, stop=True,
                                         skip_group_check=True)
                    else:
                        nc.tensor.matmul(sc[:, h2, :], kl, ql,
                                         start=True, stop=True,
                                         skip_group_check=True)
                if lo == 0:
                    nc.scalar.activation(
                        pt[:], sc[:],
                        mybir.ActivationFunctionType.Exp, scale=0.125)
                else:
                    for h2 in (0, 1):
                        nc.scalar.activation(
                            pt[:, h2, lo:512], sc[:, h2, lo:512],
                            mybir.ActivationFunctionType.Exp, scale=0.125)
                for h2 in (0, 1):
                    nc.tensor.matmul(pv[0:HD + 1, h2, lo:512], vaug[:, t, :],
                                     pt[:, h2, lo:512],
                                     start=(t == 0), stop=(t == T - 1),
                                     skip_group_check=True)
                pop_filler()
            rs = recp.tile([1, 2, 512], F32, tag="rs", name=f"rs{m}{j}")
            nc.scalar.activation(rs[:], pv[HD:HD + 1, :, :],
                                 mybir.ActivationFunctionType.Copy)
            recr = recp.tile([1, 2, 512], F32R, tag="recr", name=f"recr{m}{j}")
            nc.vector.reciprocal(recr[:], rs[:])
            for h2 in (0, 1):
                rec_b = util.tile([HD, 512], F32, tag="ut", name=f"rb{m}{j}{h2}")
                nc.tensor.matmul(rec_b[:], ones1[:], recr[:, h2, :],
                                 start=True, stop=True)
                rec_s = recp.tile([HD, 512], BF16, tag="recs",
                                  name=f"rcs{m}{j}{h2}")
                nc.vector.tensor_copy(rec_s[:], rec_b[:])
                if h2 == 0:
                    nc.vector.tensor_mul(attnT[0:HD, m, c0:c0 + 512],
                                         pv[0:HD, 0, :], rec_s[:])
                else:
                    oddt = recp.tile([HD, 512], BF16, tag="oddt",
                                     name=f"odd{m}{j}")
                    nc.vector.tensor_mul(oddt[:], pv[0:HD, 1, :], rec_s[:])
                    rp = util.tile([P, 512], F32, tag="ut", name=f"rp{m}{j}")
                    nc.tensor.matmul(rp[:], ishift_sb[:], oddt[:],
                                     start=True, stop=True)
                    nc.vector.tensor_copy(attnT[HD:P, m, c0:c0 + 512],
                                          rp[HD:P, :])

        for n in (0, 1):
            emit_proj_q(n, 0)
            emit_proj_q(n, 1)
            emit_proj_kv(n)

        fillers.extend([
            lambda: emit_proj_q(2, 0), lambda: emit_proj_q(2, 1),
            lambda: emit_proj_kv(2),
            lambda: emit_proj_q(3, 0), lambda: emit_proj_q(3, 1),
            lambda: emit_proj_kv(3),
        ])
        emit_attn(0, 0)
        emit_attn(1, 0)
        for st in range(0, 4):
            for nn in range(2):
                fillers.append(lambda st=st, nn=nn: emit_outproj(st, nn))
        emit_attn(0, 1)
        emit_attn(1, 1)
        for st in range(4, 8):
            for nn in range(2):
                fillers.append(lambda st=st, nn=nn: emit_outproj(st, nn))
        emit_attn(0, 2)
        emit_attn(1, 2)
        for st in range(8, 12):
            for nn in range(2):
                fillers.append(lambda st=st, nn=nn: emit_outproj(st, nn))
        emit_attn(0, 3)
        emit_attn(1, 3)
        while fillers:
            fillers.pop(0)()
        for st in range(12, NST):
            for nn in range(2):
                emit_outproj(st, nn)

    nc.compile()
    return nc


def make_consts():
    bf = ml_dtypes.bfloat16
    r128 = np.zeros((P, P), np.float32)
    for mm in range(P):
        hh, dd = mm // HD, mm % HD
        if dd < HD // 2:
            r128[hh * HD + dd + HD // 2, mm] = -1.0
        else:
            r128[hh * HD + dd - HD // 2, mm] = 1.0
    rdup = np.zeros((HD, P), np.float32)
    idup = np.zeros((HD, P), np.float32)
    for mm in range(P):
        dd = mm % HD
        idup[dd, mm] = 1.0
        if dd < HD // 2:
            rdup[dd + HD // 2, mm] = -1.0
        else:
            rdup[dd - HD // 2, mm] = 1.0
    ident = np.zeros((P, HD), np.float32)
    ident[HD:P, :] = np.eye(HD)
    ishift = np.zeros((HD, P), np.float32)
    for kk in range(HD):
        ishift[kk, kk + HD] = 1.0
    tri = np.where(np.arange(P)[:, None] <= np.arange(P)[None, :], 0.0,
                   MASK_NEG).astype(bf)
    i128b = np.eye(P).astype(bf)
    return dict(r128=r128.astype(bf), rdup=rdup.astype(bf),
                idup=idup.astype(bf), ident=ident.astype(bf),
                ishift=ishift.astype(bf), tri=tri, i128b=i128b,
                ones1c=np.ones((1, HD), np.float32),
                onesv=np.ones((P, NST), bf))


def make_in_maps(x, cos, sin, wq, wk, wv, wo):
    bf = ml_dtypes.bfloat16
    consts = make_consts()
    cosT = np.ascontiguousarray(np.vstack([cos.T, cos.T])).astype(bf)
    sinT = np.ascontiguousarray(np.vstack([sin.T, sin.T])).astype(bf)
    in_maps = []
    for core in range(8):
        b, g = core // NKV, core % NKV
        xT = np.ascontiguousarray(x[b].T).astype(bf)
        wqT = np.ascontiguousarray(wq[g * FEAT:(g + 1) * FEAT, :].T).astype(bf)
        wkvT = np.ascontiguousarray(
            np.concatenate([wk[g * HD:(g + 1) * HD, :],
                            wv[g * HD:(g + 1) * HD, :]], axis=0).T).astype(bf)
        woT = np.ascontiguousarray(wo[:, g * FEAT:(g + 1) * FEAT].T).astype(bf)
        in_maps.append(dict(xT=xT, wqT=wqT, wkvT=wkvT, woT=woT,
                            cosT=cosT, sinT=sinT, **consts))
    return in_maps


_PROG = None


def kernel(x, cos, sin, wq, wk, wv, wo):
    global _PROG
    x = np.asarray(x, np.float32)
    cos = np.asarray(cos, np.float32)
    sin = np.asarray(sin, np.float32)
    wq = np.asarray(wq, np.float32)
    wk = np.asarray(wk, np.float32)
    wv = np.asarray(wv, np.float32)
    wo = np.asarray(wo, np.float32)

    in_maps = make_in_maps(x, cos, sin, wq, wk, wv, wo)
    if _PROG is None:
        _PROG = build_program()
    res = run_bass_kernel_spmd(_PROG, in_maps, core_ids=list(range(8)))

    out = np.zeros((B, S, HID), np.float32)
    for core in range(8):
        out[core // NKV] += np.asarray(res.results[core]["outp"], np.float32)
    return out


if __name__ == "__main__":
    rng = np.random.default_rng(0)
    ins = dict(
        x=rng.standard_normal((B, S, HID)).astype(np.float32),
        cos=rng.random((S, HD)).astype(np.float32),
        sin=rng.random((S, HD)).astype(np.float32),
        wq=(rng.standard_normal((HID, HID)) * HID ** -0.5).astype(np.float32),
        wk=(rng.standard_normal((NKV * HD, HID)) * HID ** -0.5).astype(np.float32),
        wv=(rng.standard_normal((NKV * HD, HID)) * HID ** -0.5).astype(np.float32),
        wo=(rng.standard_normal((HID, HID)) * HID ** -0.5).astype(np.float32),
    )
    out = kernel(**ins)
    print("kernel ran, out shape", out.shape, "mean", float(np.abs(out).mean()))
